# revision 1
# baseline (speedup 1.0000x reference)
"""Multi-head attention (B=8, N=1024, C=1024, H=16) on 8 TRN2 NeuronCores.

Strategy: pure data parallelism — one batch element per core, no collectives.
Layouts avoid all on-device transposes:

  host passes    xT = x[b].T              [C, N]   (c on partitions)
                 wT = qkv_w.T             [C, 3C]  (q-columns pre-scaled)
                 pT = proj_w.T            [C, C]
  device makes   V         [n, d] natural layout, with a ones column per head
                 Q^T, K^T  [d, n] computed per head-pair (rotating pool)
                 S^T = lhsT(K^T_h) x Q^T_h   [n_k, n_q]  — the two heads of a
                     pair run CONCURRENTLY in the PE array via tile_position
                     row packing (K=64 each, rows 0-63 / 64-127)
                 P^T = exp(S^T)           (no max-subtract: |S|<=~10, safe)
                 O'^T = [V_h|1].T @ P^T   [65, n_q]  (row 64 = softmax denom Z)
                 O^T  = O'^T[0:64] * (1/Z)  (reciprocal_approx_fast; 1/Z
                     broadcast across partitions via a DRAM-bounce DMA)
                 y^T = pT.T @ O^T + b     [C, N]
  host returns   y = yT.T  per batch.

All matmuls are float32r (full-rate fp32). The per-pair software pipeline
(qk projection of pair j+1 emitted between the AV stages of pair j) keeps
the PE dense with full-array work while ACT runs the exp chains.
"""

import contextlib

import numpy as np

import concourse.bass as bass
import concourse.mybir as mybir
import concourse.tile as tile
from concourse import bacc
from concourse.bass_utils import run_bass_kernel_spmd

f32 = mybir.dt.float32
f32r = mybir.dt.float32r
EXP = mybir.ActivationFunctionType.Exp

B, N, C = 8, 1024, 1024
H, HD = 16, 64
SCALE = HD ** -0.5
NCORES = 8


def mm(nc, out, lhsT, rhs, start, stop, tile_position=None):
    nc.tensor.matmul(out, lhsT, rhs, start=start, stop=stop,
                     tile_position=tile_position)


def _rep(tc, reps):
    if reps <= 1:
        return contextlib.nullcontext()
    return tc.For_i(0, reps, 1, hint_engines=(
        mybir.EngineType.PE, mybir.EngineType.Activation,
        mybir.EngineType.DVE, mybir.EngineType.SP, mybir.EngineType.Pool))


def build(stages="ABC", reps=1):
    nc = bacc.Bacc("TRN2", target_bir_lowering=False, debug=False)
    xT = nc.dram_tensor("xT", [C, N], f32, kind="ExternalInput")
    wT = nc.dram_tensor("wT", [C, 3 * C], f32, kind="ExternalInput")
    pT = nc.dram_tensor("pT", [C, C], f32, kind="ExternalInput")
    pb = nc.dram_tensor("pb", [C, 1], f32, kind="ExternalInput")
    yT = nc.dram_tensor("yT", [C, N], f32, kind="ExternalOutput")

    with tile.TileContext(nc) as tc:
        with (
            tc.tile_pool(name="const", bufs=1) as const,
            tc.tile_pool(name="xp", bufs=8) as xp,
            tc.tile_pool(name="vpp", bufs=8) as vpp,
            tc.tile_pool(name="obp", bufs=8) as obp,
            tc.tile_pool(name="qkp", bufs=4) as qkp,
            tc.tile_pool(name="wqkp", bufs=4) as wqkp,
            tc.tile_pool(name="psQ", bufs=1, space="PSUM") as psQ,
        ):
            onesc = const.tile([128, H, 1], f32)
            nc.vector.memset(onesc, 1.0)

            xts = [xp.tile([128, N], f32r, name=f"xt{i}", tag="xt")
                   for i in range(8)]
            for ci in range(8 if stages else 0):
                # xts[0] goes on the sync queue right before the first
                # weight DMA; the rest stream in parallel on the gpsimd
                # (SWDGE) queues so the first matmul starts ~1MB of DMA in
                eng = nc.sync if ci == 0 else nc.gpsimd
                eng.dma_start(
                    out=xts[ci],
                    in_=xT[ci * 128:(ci + 1) * 128, :].bitcast(f32r))

            # V' resident: [n-chunk][128, 16 heads, 64+1]; col 64 is ones.
            vp = [vpp.tile([128, H, HD + 1], f32r, name=f"vp{i}", tag="vp")
                  for i in range(8)]
            # O^T resident: tile j = rows [j*128,(j+1)*128) = heads 2j,2j+1
            ob = [obp.tile([128, N], f32r, name=f"ob{i}", tag="ob")
                  for i in range(8)]

            def qk_chunk(d, jname):
                """Project wT columns [d*128,(d+1)*128) -> [128, N]."""
                wt = wqkp.tile([128, 8, 128], f32r,
                               name=f"wt{jname}", tag="wt")
                nc.sync.dma_start(
                    out=wt,
                    in_=wT[:, d * 128:(d + 1) * 128]
                    .rearrange("(j p) c -> p j c", p=128).bitcast(f32r))
                acc = psQ.tile([128, N], f32, name="qacc", tag="qacc")
                for qh in range(2):
                    sl = slice(qh * 512, (qh + 1) * 512)
                    for ci in range(8):
                        mm(nc, acc[:, sl], wt[:, ci, :],
                           xts[ci][:, sl],
                           start=(ci == 0), stop=(ci == 7))
                qt = qkp.tile([128, N], f32r, name=f"qk{jname}", tag="qk")
                nc.vector.tensor_copy(qt[:, :], acc[:, :])
                return qt

            def qs_stage(j):
                QT = qk_chunk(j, f"q{j}")
                KT = qk_chunk(8 + j, f"k{j}")
                return QT, KT

            # prefetch pair 0's q/k projection ahead of the V phase so the
            # PE rolls straight from V matmuls into attention
            qks = qs_stage(0) if "B" in stages else None

            # ---------------- phase V: value projection ----------------
            with (
                tc.tile_pool(name="wvp", bufs=2) as wvp,
                tc.tile_pool(name="psV", bufs=3, space="PSUM") as psV,
            ):
                with _rep(tc, reps):
                    for dv in range(2 if "A" in stages else 0):
                        wv = wvp.tile([128, 8, 512], f32r, name="wv",
                                      tag="wv")
                        nc.sync.dma_start(
                            out=wv,
                            in_=wT[:, 2048 + dv * 512:2048 + (dv + 1) * 512]
                            .rearrange("(j p) c -> p j c", p=128)
                            .bitcast(f32r))
                        for n in range(8):
                            vacc = psV.tile([128, 512], f32, name="vacc",
                                            tag="vacc")
                            for ci in range(8):
                                mm(nc, vacc[:, :],
                                   xts[ci][:, n * 128:(n + 1) * 128],
                                   wv[:, ci, :],
                                   start=(ci == 0), stop=(ci == 7))
                            nc.vector.tensor_copy(
                                vp[n][:, dv * 8:(dv + 1) * 8, 0:HD],
                                vacc.rearrange("p (g e) -> p g e", e=HD))
                    for n in range(8 if "A" in stages else 0):
                        nc.vector.tensor_copy(vp[n][:, :, HD:HD + 1],
                                              onesc[:, :, :])

            # stage-C weights/bias: allocated here so their DMAs can
            # prefetch during the attention pairs
            wpp_ctx = tc.tile_pool(name="wpp", bufs=2)
            biasp_ctx = tc.tile_pool(name="biasp", bufs=8)
            wpp = wpp_ctx.__enter__()
            biasp = biasp_ctx.__enter__()
            nC = 8 if "C" in stages else 0
            pbt = [biasp.tile([128, 1], f32, name=f"pbt{e}", tag="pbt")
                   for e in range(8)]
            for e in range(nC):
                nc.sync.dma_start(out=pbt[e],
                                  in_=pb[e * 128:(e + 1) * 128, :])

            def load_wpt(e):
                wpt = wpp.tile([128, 8, 128], f32r, name="wpt", tag="wpt")
                nc.sync.dma_start(
                    out=wpt,
                    in_=pT[:, e * 128:(e + 1) * 128]
                    .rearrange("(j p) c -> p j c", p=128).bitcast(f32r))
                return wpt

            wpts = {e: load_wpt(e) for e in range(min(nC, 2))}

            # ---------------- attention pairs (fused qk-proj + attn) -----
            with (
                tc.tile_pool(name="ppool", bufs=10) as ppool,
                tc.tile_pool(name="ovsp", bufs=2) as ovsp,
                tc.tile_pool(name="rbsp", bufs=2) as rbsp,
                tc.tile_pool(name="otp", bufs=1) as otp,
                tc.tile_pool(name="yp", bufs=2) as yp,
                tc.tile_pool(name="psSA", bufs=1, space="PSUM") as psSA,
                tc.tile_pool(name="psSB", bufs=1, space="PSUM") as psSB,
                tc.tile_pool(name="psO", bufs=2, space="PSUM") as psO,
                tc.tile_pool(name="rdr", bufs=4, space="DRAM") as rdr,
            ):
                def s_stage(j, QT, KT):
                    """Packed S^T for heads 2j (rows 0-63) and 2j+1
                    (rows 64-127): both run concurrently in the array."""
                    ptsA = [ppool.tile([128, N], f32r,
                                       name=f"pa{j}_{kc}", tag="pt")
                            for kc in range(8)]
                    ptsB = [ppool.tile([128, N], f32r,
                                       name=f"pb{j}_{kc}", tag="pt")
                            for kc in range(8)]
                    for kc in range(8):
                        stA = psSA.tile([128, N], f32, name="stA",
                                        tag="stA")
                        stB = psSB.tile([128, N], f32, name="stB",
                                        tag="stB")
                        ks = slice(kc * 128, (kc + 1) * 128)
                        for qh in range(2):
                            sl = slice(qh * 512, (qh + 1) * 512)
                            mm(nc, stA[:, sl], KT[0:64, ks],
                               QT[0:64, sl], start=True, stop=True,
                               tile_position=(0, 0))
                            mm(nc, stB[:, sl], KT[64:128, ks],
                               QT[64:128, sl], start=True, stop=True,
                               tile_position=(64, 0))
                        nc.scalar.activation(ptsA[kc][:, :], stA[:, :], EXP)
                        nc.scalar.activation(ptsB[kc][:, :], stB[:, :], EXP)
                    return ptsA, ptsB

                def av_stage(h, pts):
                    hq, ho = h // 2, (h % 2) * 64
                    ov = [psO.tile([65, 512], f32,
                                   name=f"ov{h}_{qh}", tag="ov")
                          for qh in range(2)]
                    for kc in range(8):
                        for qh in range(2):
                            sl = slice(qh * 512, (qh + 1) * 512)
                            mm(nc, ov[qh][:, :], vp[kc][:, h, :],
                               pts[kc][:, sl],
                               start=(kc == 0), stop=(kc == 7))
                    # copy O'^T out of PSUM immediately so the ov slots
                    # free for the next head's AV; then 1/Z in place on the
                    # SBUF copy and broadcast via a DRAM bounce (step-0 read)
                    ovs = ovsp.tile([65, N], f32, name=f"ovs{h}", tag="ovs")
                    for qh in range(2):
                        sl = slice(qh * 512, (qh + 1) * 512)
                        nc.vector.tensor_copy(ovs[:, sl], ov[qh][:, :])
                    rbs = rbsp.tile([64, N], f32, name="rbs", tag="rbs")
                    for qh in range(2):
                        sl = slice(qh * 512, (qh + 1) * 512)
                        nc.vector.reciprocal(ovs[64:65, sl],
                                             ovs[64:65, sl])
                        rsc = rdr.tile([1, 512], f32, name="rsc", tag="rsc")
                        nc.sync.dma_start(out=rsc, in_=ovs[64:65, sl])
                        bsrc = bass.AP(tensor=rsc.tensor, offset=rsc.offset,
                                       ap=[[0, 64], [1, 512]])
                        nc.sync.dma_start(out=rbs[:, sl], in_=bsrc)
                    if ho == 0:
                        for qh in range(2):
                            sl = slice(qh * 512, (qh + 1) * 512)
                            nc.vector.tensor_mul(ob[hq][0:64, sl],
                                                 ovs[0:64, sl],
                                                 rbs[:, sl])
                    else:
                        ot = otp.tile([64, N], f32r, name="ot", tag="ot")
                        for qh in range(2):
                            sl = slice(qh * 512, (qh + 1) * 512)
                            nc.vector.tensor_mul(ot[:, sl],
                                                 ovs[0:64, sl],
                                                 rbs[:, sl])
                        # partition shift 0-63 -> 64-127 needs a DMA
                        nc.gpsimd.dma_start(out=ob[hq][64:128, :],
                                            in_=ot[:, :])

                def proj_head(e):
                    """Stage-C chunk e, d=0..6 partial accumulation (these
                    only read ob tiles finished by earlier pairs)."""
                    wpt = wpts.pop(e, None)
                    if wpt is None:
                        wpt = load_wpt(e)
                    pool_, tag_ = ((psQ, "qacc") if e % 2 == 0
                                   else (psSA, "stA"))
                    pj = pool_.tile([128, N], f32, name="pj", tag=tag_)
                    for qh in range(2):
                        sl = slice(qh * 512, (qh + 1) * 512)
                        for d in range(7):
                            mm(nc, pj[:, sl], wpt[:, d, :],
                               ob[d][:, sl],
                               start=(d == 0), stop=False)
                    return pj, wpt

                def proj_tail(e, pj, wpt):
                    for qh in range(2):
                        sl = slice(qh * 512, (qh + 1) * 512)
                        mm(nc, pj[:, sl], wpt[:, 7, :], ob[7][:, sl],
                           start=False, stop=True)
                    yt = yp.tile([128, N], f32, name="yt", tag="yt")
                    nc.vector.tensor_scalar_add(yt[:, :], pj[:, :],
                                                pbt[e])
                    nc.sync.dma_start(
                        out=yT[e * 128:(e + 1) * 128, :], in_=yt[:, :])

                def do_proj(e):
                    proj_tail(e, *proj_head(e))

                with _rep(tc, reps):
                    npairs = 8 if "B" in stages else 0
                    pend = None
                    for j in range(npairs):
                        pj = s_stage(j, *qks)
                        if j + 1 < npairs:
                            qks = qs_stage(j + 1)
                        if pend is not None:
                            av_stage(2 * pend[0], pend[1][0])
                            av_stage(2 * pend[0] + 1, pend[1][1])
                        pend = (j, pj)
                    if pend is not None:
                        av_stage(2 * pend[0], pend[1][0])
                        # first proj chunk's d=0..6 partials emitted between
                        # the final AV stages: they fill the PE while the
                        # last exp/normalize tail drains; d=7 completes after
                        head0 = proj_head(0) if nC else None
                        av_stage(2 * pend[0] + 1, pend[1][1])
                        if head0 is not None:
                            proj_tail(0, *head0)

                # ---------- stage C: output projection (same context, so
                # no pool-release barrier between attention and proj) ----
                with _rep(tc, reps):
                    for e in range(1 if ("B" in stages and npairs) else 0,
                                   nC):
                        do_proj(e)
            biasp_ctx.__exit__(None, None, None)
            wpp_ctx.__exit__(None, None, None)
    nc.compile()
    return nc


_CACHE = {}


def _get_nc():
    if "nc" not in _CACHE:
        _CACHE["nc"] = build()
    return _CACHE["nc"]


def _prep_in_maps(x, qkv_w, proj_w, proj_b):
    wT = np.ascontiguousarray(qkv_w.T).astype(np.float32)
    wT[:, 0:C] *= np.float32(SCALE)
    pT = np.ascontiguousarray(proj_w.T).astype(np.float32)
    pbv = np.ascontiguousarray(np.asarray(proj_b, dtype=np.float32)
                               .reshape(C, 1))
    return [
        {
            "xT": np.ascontiguousarray(np.asarray(x[b], dtype=np.float32).T),
            "wT": wT,
            "pT": pT,
            "pb": pbv,
        }
        for b in range(B)
    ]


def kernel(x, qkv_w, proj_w, proj_b):
    x = np.asarray(x)
    assert x.shape == (B, N, C), x.shape
    nc = _get_nc()
    in_maps = _prep_in_maps(x, qkv_w, proj_w, proj_b)
    res = run_bass_kernel_spmd(nc, in_maps, core_ids=list(range(NCORES)))
    out = np.stack([res.results[b]["yT"].T for b in range(B)], axis=0)
    return np.ascontiguousarray(out.astype(np.float32))



# revision 32
# speedup vs baseline: 1.1417x; 1.1417x over previous
"""Multi-head attention (B=8, N=1024, C=1024, H=16) on 8 TRN2 NeuronCores.

Strategy: pure data parallelism - one batch element per core, no collectives.

  host passes    xT = x[b].T          [C, N]  f32  (c on partitions)
                 wTb = qkv_w.T        [C, 3C] bf16 (q-columns pre-scaled)
                 pT = proj_w.T        [C, C]  f32
                 pb = proj_b          [C, 1]  f32
  device makes   V    [n, head, 64+1] bf16, col 64 = ones (softmax denom)
                 Q^T, K^T             f32 [128, N] per pair (2 heads stacked)
                 KTpad: K^T zero-padded to K=128 stationaries so the
                     per-head S^T matmuls contract over 128 partitions at
                     full rate (the old tile_position row-packing ran the
                     two 64-row matmuls serially at ~2 cycles/row)
                 S^T = KTpad.T @ Q^T  [k, q] per (head, kc, qh)
                 P^T = exp(S^T)       bf16
                 O'^T = [V|1].T @ P^T [65, q]; row 64 = Z
                 1/Z via reciprocal_approx_fast on the Z row, broadcast
                     across partitions with a DRAM-bounce DMA
                 O^T = O'^T[0:64] * (1/Z)
                 y^T = pT.T @ O^T + b [C, N]
  host returns   y = yT.T per batch.

Engine split: PE matmuls; ACT exp; DVE reciprocal+normalize muls+QK casts;
Pool (nc.gpsimd) PSUM evacuation of O' and V, bias adds; DMA issue spread
over sync/scalar/vector/gpsimd queues so no single queue serializes.
"""

from collections import deque

import numpy as np

import concourse.bass as bass
import concourse.mybir as mybir
import concourse.tile as tile
from concourse import bacc
from concourse.bass_utils import run_bass_kernel_spmd

f32 = mybir.dt.float32
f32r = mybir.dt.float32r
bf16 = mybir.dt.bfloat16
EXP = mybir.ActivationFunctionType.Exp

B, N, C = 8, 1024, 1024
H, HD = 16, 64
SCALE = HD ** -0.5
NCORES = 8


def build():
    nc = bacc.Bacc("TRN2", target_bir_lowering=False, debug=False)
    xT = nc.dram_tensor("xT", [C, N], f32, kind="ExternalInput")
    wT = nc.dram_tensor("wT", [C, 3 * C], f32, kind="ExternalInput")
    pTb = nc.dram_tensor("pTb", [C, C], bf16, kind="ExternalInput")
    pb = nc.dram_tensor("pb", [C, 1], f32, kind="ExternalInput")
    yT = nc.dram_tensor("yT", [C, N], f32, kind="ExternalOutput")

    with tile.TileContext(nc) as tc:
        with (
            tc.tile_pool(name="const", bufs=1) as const,
            tc.tile_pool(name="xp", bufs=8) as xp,
            tc.tile_pool(name="vpp", bufs=8) as vpp,
            tc.tile_pool(name="obp", bufs=8) as obp,
            tc.tile_pool(name="qtp", bufs=2) as qtp,
            tc.tile_pool(name="wqkp", bufs=4) as wqkp,
            tc.tile_pool(name="wpp", bufs=8) as wpp,
            tc.tile_pool(name="ptsp", bufs=12) as ptsp,
            tc.tile_pool(name="ovsp", bufs=4) as ovsp,
            tc.tile_pool(name="rzp", bufs=2) as rzp,
            tc.tile_pool(name="otp", bufs=2) as otp,
            tc.tile_pool(name="ytp", bufs=2) as ytp,
            tc.tile_pool(name="psQ", bufs=1, space="PSUM") as psQ,
            tc.tile_pool(name="rdr", bufs=8, space="DRAM") as rdr,
        ):
            # ---- bias: one DMA into [128, 8]; column e = chunk e's bias
            pbt = const.tile([128, 8], f32)
            nc.sync.dma_start(
                out=pbt, in_=pb.rearrange("(e p) x -> p (e x)", p=128))

            # ---- x tiles: xT row chunks, f32r
            xts = [xp.tile([128, N], f32r, name=f"xt{i}", tag="xt")
                   for i in range(8)]
            for ci in range(8):
                eng = nc.sync if ci == 0 else nc.gpsimd
                eng.dma_start(
                    out=xts[ci],
                    in_=xT[ci * 128:(ci + 1) * 128, :].bitcast(f32r))

            # ---- V resident: [n-chunk][128, 16 heads, 64+1] bf16
            vp = [vpp.tile([128, H, HD + 1], bf16, name=f"vp{i}", tag="vp")
                  for i in range(8)]
            for n in range(8):
                nc.vector.memset(vp[n][:, :, HD:HD + 1], 1.0)

            # ---- O^T resident: tile hq = rows [hq*128,(hq+1)*128) of O^T
            ob = [obp.tile([128, N], bf16, name=f"ob{i}", tag="ob")
                  for i in range(8)]

            # ---- proj weights: tiles now, DMAs staggered one per pair start
            wpts = [wpp.tile([128, 8, 128], bf16, name=f"wpt{e}", tag="wpt")
                    for e in range(8)]

            def wpt_dma(e):
                nc.scalar.dma_start(
                    out=wpts[e],
                    in_=pTb[:, e * 128:(e + 1) * 128]
                    .rearrange("(j p) c -> p j c", p=128))

            # ---- KTpad stationaries: two rotating sets, dead halves zeroed
            # once (never rewritten, so zeros persist across pair reuse)
            ktpA = [const.tile([128, N], f32r, name=f"ktpA{i}")
                    for i in range(2)]
            ktpB = [const.tile([128, N], f32r, name=f"ktpB{i}")
                    for i in range(2)]
            zc = const.tile([128, 512], f32, name="zc")
            nc.vector.memset(zc, 0.0)
            for i in range(2):
                for qh in range(2):
                    sl = slice(qh * 512, (qh + 1) * 512)
                    nc.vector.tensor_copy(ktpA[i][64:128, sl],
                                          zc[64:128, :])
                    nc.vector.tensor_copy(ktpB[i][0:64, sl], zc[0:64, :])

            # ---- qk projection chunk -> psQ accumulation (thunk-based)
            def qk_mm_thunks(d, dma_eng):
                """DMA the wT column chunk d now; return matmul thunks that
                accumulate [128, N] into a fresh psQ tile + the acc tile."""
                wt = wqkp.tile([128, 8, 128], f32r, name=f"wt{d}", tag="wt")
                dma_eng.dma_start(
                    out=wt,
                    in_=wT[:, d * 128:(d + 1) * 128]
                    .rearrange("(j p) c -> p j c", p=128).bitcast(f32r))
                acc = psQ.tile([128, N], f32, name=f"qacc{d}", tag="qacc")
                thunks = []
                for ci in range(8):
                    for qh in range(2):
                        sl = slice(qh * 512, (qh + 1) * 512)

                        def t(ci=ci, sl=sl, acc=acc, wt=wt):
                            nc.tensor.matmul(
                                acc[:, sl], wt[:, ci, :], xts[ci][:, sl],
                                start=(ci == 0), stop=(ci == 7))
                        thunks.append(t)
                return acc, thunks

            fill = deque()

            def pump(k):
                for _ in range(k):
                    if fill:
                        fill.popleft()()

            def flush():
                while fill:
                    fill.popleft()()

            # ---- startup: pair-0 Q and K chunks emitted directly
            accQ, tQ = qk_mm_thunks(0, nc.sync)
            for t in tQ:
                t()
            qt0 = qtp.tile([128, N], f32r, name="qt0", tag="qt")
            nc.vector.tensor_copy(qt0, accQ)
            accK, tK = qk_mm_thunks(8, nc.sync)
            for t in tK:
                t()
            nc.vector.tensor_copy(ktpA[0][0:64, :], accK[0:64, :])
            nc.vector.tensor_copy(ktpB[0][64:128, :], accK[64:128, :])

            # ---- phase V: value projection (scoped PSUM pool)
            with (
                tc.tile_pool(name="psV", bufs=3, space="PSUM") as psV,
                tc.tile_pool(name="wvp", bufs=2) as wvp,
            ):
                for dv in range(2):
                    wv = wvp.tile([128, 8, 512], f32r,
                                  name=f"wv{dv}", tag="wv")
                    nc.scalar.dma_start(
                        out=wv,
                        in_=wT[:, 2048 + dv * 512:2048 + (dv + 1) * 512]
                        .rearrange("(j p) c -> p j c", p=128).bitcast(f32r))
                    for n in range(8):
                        vacc = psV.tile([128, 512], f32, name="vacc",
                                        tag="vacc")
                        for ci in range(8):
                            nc.tensor.matmul(
                                vacc[:, :],
                                xts[ci][:, n * 128:(n + 1) * 128],
                                wv[:, ci, :],
                                start=(ci == 0), stop=(ci == 7))
                        nc.scalar.activation(
                            vp[n][:, dv * 8:(dv + 1) * 8, 0:HD],
                            vacc.rearrange("p (g e) -> p g e", e=HD),
                            mybir.ActivationFunctionType.Copy)

            # ---- attention pairs ------------------------------------
            with (
                tc.tile_pool(name="psS", bufs=2, space="PSUM") as psS,
                tc.tile_pool(name="psO", bufs=4, space="PSUM") as psO,
            ):
                def av_mm(ovt, h, kc, qh, pts):
                    sl = slice(qh * 512, (qh + 1) * 512)
                    nc.tensor.matmul(
                        ovt[:, :], vp[kc][:, h, :], pts[kc][:, sl],
                        start=(kc == 0), stop=(kc == 7))

                def norm(h, ov2):
                    """O = O' * 1/Z for head h; writes ob in the right rows."""
                    hq, odd = h // 2, h % 2
                    rz = rzp.tile([64, N], f32, name=f"rz{h}", tag="rz")
                    ovss = []
                    for qh in range(2):
                        ovs = ovsp.tile([65, 512], f32, name=f"ovs{h}_{qh}",
                                        tag="ovs")
                        nc.scalar.activation(
                            ovs, ov2[qh], mybir.ActivationFunctionType.Copy)
                        zdr = rdr.tile([1, 512], f32, name="zd", tag="zd")
                        # write + broadcast-read share the sync queue: the
                        # DRAM bounce tile is not dep-tracked, FIFO order is
                        nc.sync.dma_start(out=zdr, in_=ovs[64:65, :])
                        bsrc = bass.AP(tensor=zdr.tensor, offset=zdr.offset,
                                       ap=[[0, 64], [1, 512]])
                        nc.sync.dma_start(
                            out=rz[:, qh * 512:(qh + 1) * 512], in_=bsrc)
                        ovss.append(ovs)
                    # approx-fast reciprocal only works at partition base 0,
                    # so invert the broadcast copy, not the PSUM Z row
                    nc.vector.reciprocal_approx_fast(rz[:, :], rz[:, :])
                    if not odd:
                        for qh in range(2):
                            sl = slice(qh * 512, (qh + 1) * 512)
                            nc.vector.tensor_mul(
                                ob[hq][0:64, sl], ovss[qh][0:64, :],
                                rz[:, sl])
                    else:
                        ot = otp.tile([64, N], bf16, name=f"ot{h}", tag="ot")
                        for qh in range(2):
                            sl = slice(qh * 512, (qh + 1) * 512)
                            nc.vector.tensor_mul(
                                ot[:, sl], ovss[qh][0:64, :], rz[:, sl])
                        nc.gpsimd.dma_start(out=ob[hq][64:128, :],
                                            in_=ot[:, :])

                QT = qt0
                ktA, ktB = ktpA[0], ktpB[0]

                # proj helpers (emitted at/after pair 7)
                def proj_mm(pjq, e, d, qh):
                    sl = slice(qh * 512, (qh + 1) * 512)
                    nc.tensor.matmul(
                        pjq[qh][:, :] if isinstance(pjq, list)
                        else pjq[:, sl],
                        wpts[e][:, d, :], ob[d][:, sl],
                        start=(d == 0), stop=(d == 7))

                def proj_drain(pj, e):
                    yt = ytp.tile([128, N], f32, name=f"yt{e}", tag="yt")
                    if isinstance(pj, list):
                        for qh in range(2):
                            sl = slice(qh * 512, (qh + 1) * 512)
                            nc.vector.tensor_scalar_add(
                                yt[:, sl], pj[qh][:, :], pbt[:, e:e + 1])
                    else:
                        nc.vector.tensor_scalar_add(yt[:, :], pj[:, :],
                                                    pbt[:, e:e + 1])
                    nc.sync.dma_start(
                        out=yT[e * 128:(e + 1) * 128, :], in_=yt[:, :])

                pj0 = None
                for j in range(8):
                    hE, hO = 2 * j, 2 * j + 1
                    wpt_dma(j)
                    ptsE = [ptsp.tile([128, N], bf16, name=f"pe{j}_{k}",
                                      tag="pts") for k in range(8)]
                    ptsO = [ptsp.tile([128, N], bf16, name=f"po{j}_{k}",
                                      tag="pts") for k in range(8)]
                    # queue next pair's qk work as PE filler
                    if j < 7:
                        accQn, tQn = qk_mm_thunks(j + 1, nc.scalar)
                        qtn = qtp.tile([128, N], f32r, name=f"qt{j+1}",
                                       tag="qt")
                        fill.extend(tQn)
                        fill.append(lambda q=qtn, a=accQn:
                                    nc.vector.tensor_copy(q, a))
                        accKn, tKn = qk_mm_thunks(9 + j, nc.scalar)
                        fill.extend(tKn)
                        s = (j + 1) % 2

                        def kcast(a=accKn, s=s):
                            nc.vector.tensor_copy(ktpA[s][0:64, :],
                                                  a[0:64, :])
                            nc.vector.tensor_copy(ktpB[s][64:128, :],
                                                  a[64:128, :])
                        fill.append(kcast)
                        QTn = qtn
                    else:
                        # pair 7: chunk e=0 proj partials (d=0..6) as filler
                        pj0 = psQ.tile([128, N], f32, name="pj0", tag="qacc")
                        for d in range(7):
                            for qh in range(2):
                                fill.append(lambda d=d, qh=qh:
                                            proj_mm(pj0, 0, d, qh))

                    ovE = [psO.tile([65, 512], f32, name=f"ovE{j}_{q}",
                                    tag="ov") for q in range(2)]
                    ovO = [psO.tile([65, 512], f32, name=f"ovO{j}_{q}",
                                    tag="ov") for q in range(2)]

                    for kc in range(8):
                        ks = slice(kc * 128, (kc + 1) * 128)
                        for qh in range(2):
                            sl = slice(qh * 512, (qh + 1) * 512)
                            st = psS.tile([128, 512], f32, name="st",
                                          tag="st")
                            nc.tensor.matmul(st[:, :], ktA[:, ks],
                                             QT[:, sl], start=True,
                                             stop=True)
                            nc.scalar.activation(ptsE[kc][:, sl], st[:, :],
                                                 EXP)
                        pump(2)
                        for qh in range(2):
                            sl = slice(qh * 512, (qh + 1) * 512)
                            st = psS.tile([128, 512], f32, name="st",
                                          tag="st")
                            nc.tensor.matmul(st[:, :], ktB[:, ks],
                                             QT[:, sl], start=True,
                                             stop=True)
                            nc.scalar.activation(ptsO[kc][:, sl], st[:, :],
                                                 EXP)
                        pump(2)
                        if kc >= 1:
                            for qh in range(2):
                                av_mm(ovE[qh], hE, kc - 1, qh, ptsE)
                            pump(1)
                            for qh in range(2):
                                av_mm(ovO[qh], hO, kc - 1, qh, ptsO)
                            pump(1)
                    # last AV chunk + normalization chains
                    for qh in range(2):
                        av_mm(ovE[qh], hE, 7, qh, ptsE)
                    norm(hE, ovE)
                    for qh in range(2):
                        av_mm(ovO[qh], hO, 7, qh, ptsO)
                    norm(hO, ovO)
                    if j < 7:
                        # leftover fillers include the next pair's KT cast;
                        # they must be emitted before pair j+1 reads ktA/ktB
                        flush()
                        QT, ktA, ktB = QTn, ktpA[(j + 1) % 2], \
                            ktpB[(j + 1) % 2]

                flush()
                # chunk 0: d=7 taps + drain
                for qh in range(2):
                    proj_mm(pj0, 0, 7, qh)
                proj_drain(pj0, 0)
                # chunks 1..7, two in flight (psQ tile / psS tile pair)
                for e in range(1, 8):
                    if e % 2 == 0:
                        pj = psQ.tile([128, N], f32, name=f"pj{e}",
                                      tag="qacc")
                    else:
                        pj = [psS.tile([128, 512], f32, name=f"pj{e}_{q}",
                                       tag="st") for q in range(2)]
                    for d in range(8):
                        for qh in range(2):
                            proj_mm(pj, e, d, qh)
                    proj_drain(pj, e)
    nc.compile()
    return nc


_CACHE = {}


def _get_nc():
    if "nc" not in _CACHE:
        _CACHE["nc"] = build()
    return _CACHE["nc"]


def _prep_in_maps(x, qkv_w, proj_w, proj_b):
    import ml_dtypes
    wT = np.ascontiguousarray(np.asarray(qkv_w, dtype=np.float32).T).copy()
    wT[:, 0:C] *= np.float32(SCALE)
    pTb = np.ascontiguousarray(np.asarray(proj_w, dtype=np.float32).T) \
        .astype(ml_dtypes.bfloat16)
    pbv = np.ascontiguousarray(np.asarray(proj_b, dtype=np.float32)
                               .reshape(C, 1))
    return [
        {
            "xT": np.ascontiguousarray(np.asarray(x[b], dtype=np.float32).T),
            "wT": wT,
            "pTb": pTb,
            "pb": pbv,
        }
        for b in range(B)
    ]


def kernel(x, qkv_w, proj_w, proj_b):
    x = np.asarray(x)
    assert x.shape == (B, N, C), x.shape
    nc = _get_nc()
    in_maps = _prep_in_maps(x, qkv_w, proj_w, proj_b)
    res = run_bass_kernel_spmd(nc, in_maps, core_ids=list(range(NCORES)))
    out = np.stack([res.results[b]["yT"].T for b in range(B)], axis=0)
    return np.ascontiguousarray(out.astype(np.float32))


# revision 33
# speedup vs baseline: 1.2465x; 1.0919x over previous
"""Multi-head attention (B=8, N=1024, C=1024, H=16) on 8 TRN2 NeuronCores.

Strategy: pure data parallelism - one batch element per core, no collectives.

  host passes    xT = x[b].T          [C, N]  f32  (c on partitions)
                 wTb = qkv_w.T        [C, 3C] bf16 (q-columns pre-scaled)
                 pT = proj_w.T        [C, C]  f32
                 pb = proj_b          [C, 1]  f32
  device makes   V    [n, head, 64+1] bf16, col 64 = ones (softmax denom)
                 Q^T, K^T             f32 [128, N] per pair (2 heads stacked)
                 KTpad: K^T zero-padded to K=128 stationaries so the
                     per-head S^T matmuls contract over 128 partitions at
                     full rate (the old tile_position row-packing ran the
                     two 64-row matmuls serially at ~2 cycles/row)
                 S^T = KTpad.T @ Q^T  [k, q] per (head, kc, qh)
                 P^T = exp(S^T)       bf16
                 O'^T = [V|1].T @ P^T [65, q]; row 64 = Z
                 1/Z via reciprocal_approx_fast on the Z row, broadcast
                     across partitions with a DRAM-bounce DMA
                 O^T = O'^T[0:64] * (1/Z)
                 y^T = pT.T @ O^T + b [C, N]
  host returns   y = yT.T per batch.

Engine split: PE matmuls; ACT exp; DVE reciprocal+normalize muls+QK casts;
Pool (nc.gpsimd) PSUM evacuation of O' and V, bias adds; DMA issue spread
over sync/scalar/vector/gpsimd queues so no single queue serializes.
"""

from collections import deque

import numpy as np

import concourse.bass as bass
import concourse.mybir as mybir
import concourse.tile as tile
from concourse import bacc
from concourse.bass_utils import run_bass_kernel_spmd

f32 = mybir.dt.float32
f32r = mybir.dt.float32r
bf16 = mybir.dt.bfloat16
EXP = mybir.ActivationFunctionType.Exp

B, N, C = 8, 1024, 1024
H, HD = 16, 64
SCALE = HD ** -0.5
NCORES = 8


def build():
    nc = bacc.Bacc("TRN2", target_bir_lowering=False, debug=False)
    xTb = nc.dram_tensor("xTb", [C, N], bf16, kind="ExternalInput")
    wTb = nc.dram_tensor("wTb", [C, 3 * C], bf16, kind="ExternalInput")
    pTb = nc.dram_tensor("pTb", [C, C], bf16, kind="ExternalInput")
    pb = nc.dram_tensor("pb", [C, 1], f32, kind="ExternalInput")
    yT = nc.dram_tensor("yT", [C, N], f32, kind="ExternalOutput")

    with tile.TileContext(nc) as tc:
        with (
            tc.tile_pool(name="const", bufs=1) as const,
            tc.tile_pool(name="xp", bufs=8) as xp,
            tc.tile_pool(name="vpp", bufs=8) as vpp,
            tc.tile_pool(name="obp", bufs=8) as obp,
            tc.tile_pool(name="qtp", bufs=2) as qtp,
            tc.tile_pool(name="wqkp", bufs=4) as wqkp,
            tc.tile_pool(name="wpp", bufs=8) as wpp,
            tc.tile_pool(name="ptsp", bufs=12) as ptsp,
            tc.tile_pool(name="ovsp", bufs=4) as ovsp,
            tc.tile_pool(name="rzp", bufs=2) as rzp,
            tc.tile_pool(name="otp", bufs=2) as otp,
            tc.tile_pool(name="ytp", bufs=2) as ytp,
            tc.tile_pool(name="psQ", bufs=1, space="PSUM") as psQ,
            tc.tile_pool(name="rdr", bufs=8, space="DRAM") as rdr,
        ):
            # ---- bias: one DMA into [128, 8]; column e = chunk e's bias
            pbt = const.tile([128, 8], f32)
            nc.sync.dma_start(
                out=pbt, in_=pb.rearrange("(e p) x -> p (e x)", p=128))

            # ---- x tiles: xT row chunks, f32r
            xts = [xp.tile([128, N], bf16, name=f"xt{i}", tag="xt")
                   for i in range(8)]
            for ci in range(8):
                eng = nc.sync if ci == 0 else nc.gpsimd
                eng.dma_start(
                    out=xts[ci],
                    in_=xTb[ci * 128:(ci + 1) * 128, :])

            # ---- V resident: [n-chunk][128, 16 heads, 64+1] bf16
            vp = [vpp.tile([128, H, HD + 1], bf16, name=f"vp{i}", tag="vp")
                  for i in range(8)]
            for n in range(8):
                nc.vector.memset(vp[n][:, :, HD:HD + 1], 1.0)

            # ---- O^T resident: tile hq = rows [hq*128,(hq+1)*128) of O^T
            ob = [obp.tile([128, N], bf16, name=f"ob{i}", tag="ob")
                  for i in range(8)]

            # ---- proj weights: tiles now, DMAs staggered one per pair start
            wpts = [wpp.tile([128, 8, 128], bf16, name=f"wpt{e}", tag="wpt")
                    for e in range(8)]

            def wpt_dma(e):
                nc.scalar.dma_start(
                    out=wpts[e],
                    in_=pTb[:, e * 128:(e + 1) * 128]
                    .rearrange("(j p) c -> p j c", p=128))

            # ---- KTpad stationaries: two rotating sets, dead halves zeroed
            # once (never rewritten, so zeros persist across pair reuse)
            ktpA = [const.tile([128, N], f32r, name=f"ktpA{i}")
                    for i in range(2)]
            ktpB = [const.tile([128, N], f32r, name=f"ktpB{i}")
                    for i in range(2)]
            zc = const.tile([128, 512], f32, name="zc")
            nc.vector.memset(zc, 0.0)
            for i in range(2):
                for qh in range(2):
                    sl = slice(qh * 512, (qh + 1) * 512)
                    nc.vector.tensor_copy(ktpA[i][64:128, sl],
                                          zc[64:128, :])
                    nc.vector.tensor_copy(ktpB[i][0:64, sl], zc[0:64, :])

            # ---- qk projection chunk -> psQ accumulation (thunk-based)
            def qk_mm_thunks(d, dma_eng):
                """DMA the wT column chunk d now; return matmul thunks that
                accumulate [128, N] into a fresh psQ tile + the acc tile."""
                wt = wqkp.tile([128, 8, 128], bf16, name=f"wt{d}", tag="wt")
                dma_eng.dma_start(
                    out=wt,
                    in_=wTb[:, d * 128:(d + 1) * 128]
                    .rearrange("(j p) c -> p j c", p=128))
                acc = psQ.tile([128, N], f32, name=f"qacc{d}", tag="qacc")
                thunks = []
                for ci in range(8):
                    for qh in range(2):
                        sl = slice(qh * 512, (qh + 1) * 512)

                        def t(ci=ci, sl=sl, acc=acc, wt=wt):
                            nc.tensor.matmul(
                                acc[:, sl], wt[:, ci, :], xts[ci][:, sl],
                                start=(ci == 0), stop=(ci == 7))
                        thunks.append(t)
                return acc, thunks

            fill = deque()

            def pump(k):
                for _ in range(k):
                    if fill:
                        fill.popleft()()

            def flush():
                while fill:
                    fill.popleft()()

            # ---- startup: pair-0 Q and K chunks emitted directly
            accQ, tQ = qk_mm_thunks(0, nc.sync)
            for t in tQ:
                t()
            qt0 = qtp.tile([128, N], f32r, name="qt0", tag="qt")
            nc.vector.tensor_copy(qt0, accQ)
            accK, tK = qk_mm_thunks(8, nc.sync)
            for t in tK:
                t()
            nc.vector.tensor_copy(ktpA[0][0:64, :], accK[0:64, :])
            nc.vector.tensor_copy(ktpB[0][64:128, :], accK[64:128, :])

            # ---- phase V: value projection (scoped PSUM pool)
            with (
                tc.tile_pool(name="psV", bufs=3, space="PSUM") as psV,
                tc.tile_pool(name="wvp", bufs=2) as wvp,
            ):
                for dv in range(2):
                    wv = wvp.tile([128, 8, 512], bf16,
                                  name=f"wv{dv}", tag="wv")
                    nc.gpsimd.dma_start(
                        out=wv,
                        in_=wTb[:, 2048 + dv * 512:2048 + (dv + 1) * 512]
                        .rearrange("(j p) c -> p j c", p=128))
                    for n in range(8):
                        vacc = psV.tile([128, 512], f32, name="vacc",
                                        tag="vacc")
                        for ci in range(8):
                            nc.tensor.matmul(
                                vacc[:, :],
                                xts[ci][:, n * 128:(n + 1) * 128],
                                wv[:, ci, :],
                                start=(ci == 0), stop=(ci == 7))
                        nc.scalar.activation(
                            vp[n][:, dv * 8:(dv + 1) * 8, 0:HD],
                            vacc.rearrange("p (g e) -> p g e", e=HD),
                            mybir.ActivationFunctionType.Copy)

            # ---- attention pairs ------------------------------------
            with (
                tc.tile_pool(name="psS", bufs=2, space="PSUM") as psS,
                tc.tile_pool(name="psO", bufs=4, space="PSUM") as psO,
            ):
                def av_mm(ovt, h, kc, qh, pts):
                    sl = slice(qh * 512, (qh + 1) * 512)
                    nc.tensor.matmul(
                        ovt[:, :], vp[kc][:, h, :], pts[kc][:, sl],
                        start=(kc == 0), stop=(kc == 7))

                def norm(h, ov2, tail=False):
                    """O = O' * 1/Z for head h; writes ob in the right rows."""
                    hq, odd = h // 2, h % 2
                    rz = rzp.tile([64, N], f32, name=f"rz{h}", tag="rz")
                    ovss = []
                    for qh in range(2):
                        ovs = ovsp.tile([65, 512], f32, name=f"ovs{h}_{qh}",
                                        tag="ovs")
                        if tail:
                            nc.scalar.activation(
                                ovs, ov2[qh],
                                mybir.ActivationFunctionType.Copy)
                        else:
                            nc.vector.tensor_copy(ovs, ov2[qh])
                        zdr = rdr.tile([1, 512], f32, name="zd", tag="zd")
                        # write + broadcast-read share the sync queue: the
                        # DRAM bounce tile is not dep-tracked, FIFO order is
                        nc.sync.dma_start(out=zdr, in_=ovs[64:65, :])
                        bsrc = bass.AP(tensor=zdr.tensor, offset=zdr.offset,
                                       ap=[[0, 64], [1, 512]])
                        nc.sync.dma_start(
                            out=rz[:, qh * 512:(qh + 1) * 512], in_=bsrc)
                        ovss.append(ovs)
                    # approx-fast reciprocal only works at partition base 0,
                    # so invert the broadcast copy, not the PSUM Z row
                    nc.vector.reciprocal_approx_fast(rz[:, :], rz[:, :])
                    if not odd:
                        for qh in range(2):
                            sl = slice(qh * 512, (qh + 1) * 512)
                            nc.vector.tensor_mul(
                                ob[hq][0:64, sl], ovss[qh][0:64, :],
                                rz[:, sl])
                    else:
                        ot = otp.tile([64, N], bf16, name=f"ot{h}", tag="ot")
                        for qh in range(2):
                            sl = slice(qh * 512, (qh + 1) * 512)
                            nc.vector.tensor_mul(
                                ot[:, sl], ovss[qh][0:64, :], rz[:, sl])
                        nc.gpsimd.dma_start(out=ob[hq][64:128, :],
                                            in_=ot[:, :])

                QT = qt0
                ktA, ktB = ktpA[0], ktpB[0]

                # proj helpers (emitted at/after pair 7)
                def proj_mm(pjq, e, d, qh):
                    sl = slice(qh * 512, (qh + 1) * 512)
                    nc.tensor.matmul(
                        pjq[qh][:, :] if isinstance(pjq, list)
                        else pjq[:, sl],
                        wpts[e][:, d, :], ob[d][:, sl],
                        start=(d == 0), stop=(d == 7))

                def proj_drain(pj, e):
                    yt = ytp.tile([128, N], f32, name=f"yt{e}", tag="yt")
                    if isinstance(pj, list):
                        for qh in range(2):
                            sl = slice(qh * 512, (qh + 1) * 512)
                            nc.vector.tensor_scalar_add(
                                yt[:, sl], pj[qh][:, :], pbt[:, e:e + 1])
                    else:
                        nc.vector.tensor_scalar_add(yt[:, :], pj[:, :],
                                                    pbt[:, e:e + 1])
                    nc.gpsimd.dma_start(
                        out=yT[e * 128:(e + 1) * 128, :], in_=yt[:, :])

                pj0 = None
                for j in range(8):
                    hE, hO = 2 * j, 2 * j + 1
                    wpt_dma(j)
                    ptsE = [ptsp.tile([128, N], bf16, name=f"pe{j}_{k}",
                                      tag="pts") for k in range(8)]
                    ptsO = [ptsp.tile([128, N], bf16, name=f"po{j}_{k}",
                                      tag="pts") for k in range(8)]
                    # queue next pair's qk work as PE filler
                    if j < 7:
                        accQn, tQn = qk_mm_thunks(j + 1, nc.scalar)
                        qtn = qtp.tile([128, N], f32r, name=f"qt{j+1}",
                                       tag="qt")
                        fill.extend(tQn)
                        fill.append(lambda q=qtn, a=accQn:
                                    nc.vector.tensor_copy(q, a))
                        accKn, tKn = qk_mm_thunks(9 + j, nc.scalar)
                        fill.extend(tKn)
                        s = (j + 1) % 2

                        def kcast(a=accKn, s=s):
                            nc.vector.tensor_copy(ktpA[s][0:64, :],
                                                  a[0:64, :])
                            nc.vector.tensor_copy(ktpB[s][64:128, :],
                                                  a[64:128, :])
                        fill.append(kcast)
                        QTn = qtn
                    else:
                        # pair 7: chunk e=0 proj partials (d=0..6) as filler
                        pj0 = psQ.tile([128, N], f32, name="pj0", tag="qacc")
                        for d in range(7):
                            for qh in range(2):
                                fill.append(lambda d=d, qh=qh:
                                            proj_mm(pj0, 0, d, qh))

                    ovE = [psO.tile([65, 512], f32, name=f"ovE{j}_{q}",
                                    tag="ov") for q in range(2)]
                    ovO = [psO.tile([65, 512], f32, name=f"ovO{j}_{q}",
                                    tag="ov") for q in range(2)]

                    for kc in range(8):
                        ks = slice(kc * 128, (kc + 1) * 128)
                        for qh in range(2):
                            sl = slice(qh * 512, (qh + 1) * 512)
                            st = psS.tile([128, 512], f32, name="st",
                                          tag="st")
                            nc.tensor.matmul(st[:, :], ktA[:, ks],
                                             QT[:, sl], start=True,
                                             stop=True)
                            nc.scalar.activation(ptsE[kc][:, sl], st[:, :],
                                                 EXP)
                        pump(2)
                        if kc >= 1:
                            for qh in range(2):
                                av_mm(ovE[qh], hE, kc - 1, qh, ptsE)
                            pump(1)
                        for qh in range(2):
                            sl = slice(qh * 512, (qh + 1) * 512)
                            st = psS.tile([128, 512], f32, name="st",
                                          tag="st")
                            nc.tensor.matmul(st[:, :], ktB[:, ks],
                                             QT[:, sl], start=True,
                                             stop=True)
                            nc.scalar.activation(ptsO[kc][:, sl], st[:, :],
                                                 EXP)
                        pump(2)
                        if kc >= 1:
                            for qh in range(2):
                                av_mm(ovO[qh], hO, kc - 1, qh, ptsO)
                            pump(1)
                    # last AV chunk + normalization chains
                    for qh in range(2):
                        av_mm(ovE[qh], hE, 7, qh, ptsE)
                    norm(hE, ovE, tail=(j == 7))
                    for qh in range(2):
                        av_mm(ovO[qh], hO, 7, qh, ptsO)
                    norm(hO, ovO, tail=(j == 7))
                    if j < 7:
                        # leftover fillers include the next pair's KT cast;
                        # they must be emitted before pair j+1 reads ktA/ktB
                        flush()
                        QT, ktA, ktB = QTn, ktpA[(j + 1) % 2], \
                            ktpB[(j + 1) % 2]

                flush()
                # e1 partials cover the last norm-chain latency before the
                # ob[7]-gated d=7 taps
                pj1 = [psS.tile([128, 512], f32, name=f"pj1_{q}", tag="st")
                       for q in range(2)]
                for d in range(7):
                    for qh in range(2):
                        proj_mm(pj1, 1, d, qh)
                for qh in range(2):
                    proj_mm(pj0, 0, 7, qh)
                proj_drain(pj0, 0)
                for qh in range(2):
                    proj_mm(pj1, 1, 7, qh)
                proj_drain(pj1, 1)
                # chunks 2..7, two in flight (psQ tile / psS tile pair)
                for e in range(2, 8):
                    if e % 2 == 0:
                        pj = psQ.tile([128, N], f32, name=f"pj{e}",
                                      tag="qacc")
                    else:
                        pj = [psS.tile([128, 512], f32, name=f"pj{e}_{q}",
                                       tag="st") for q in range(2)]
                    for d in range(8):
                        for qh in range(2):
                            proj_mm(pj, e, d, qh)
                    proj_drain(pj, e)
    nc.compile()
    return nc


_CACHE = {}


def _get_nc():
    if "nc" not in _CACHE:
        _CACHE["nc"] = build()
    return _CACHE["nc"]


def _prep_in_maps(x, qkv_w, proj_w, proj_b):
    import ml_dtypes
    wT = np.ascontiguousarray(np.asarray(qkv_w, dtype=np.float32).T).copy()
    wT[:, 0:C] *= np.float32(SCALE)
    pTb = np.ascontiguousarray(np.asarray(proj_w, dtype=np.float32).T) \
        .astype(ml_dtypes.bfloat16)
    pbv = np.ascontiguousarray(np.asarray(proj_b, dtype=np.float32)
                               .reshape(C, 1))
    wTb = wT.astype(ml_dtypes.bfloat16)
    return [
        {
            "xTb": np.ascontiguousarray(np.asarray(x[b], dtype=np.float32).T)
            .astype(ml_dtypes.bfloat16),
            "wTb": wTb,
            "pTb": pTb,
            "pb": pbv,
        }
        for b in range(B)
    ]


def kernel(x, qkv_w, proj_w, proj_b):
    x = np.asarray(x)
    assert x.shape == (B, N, C), x.shape
    nc = _get_nc()
    in_maps = _prep_in_maps(x, qkv_w, proj_w, proj_b)
    res = run_bass_kernel_spmd(nc, in_maps, core_ids=list(range(NCORES)))
    out = np.stack([res.results[b]["yT"].T for b in range(B)], axis=0)
    return np.ascontiguousarray(out.astype(np.float32))


# revision 34
# speedup vs baseline: 1.2980x; 1.0413x over previous
"""Multi-head attention (B=8, N=1024, C=1024, H=16) on 8 TRN2 NeuronCores.

Strategy: pure data parallelism - one batch element per core, no collectives.

  host passes    xT = x[b].T          [C, N]  f32  (c on partitions)
                 wTb = qkv_w.T        [C, 3C] bf16 (q-columns pre-scaled)
                 pT = proj_w.T        [C, C]  f32
                 pb = proj_b          [C, 1]  f32
  device makes   V    [n, head, 64+1] bf16, col 64 = ones (softmax denom)
                 Q^T, K^T             f32 [128, N] per pair (2 heads stacked)
                 KTpad: K^T zero-padded to K=128 stationaries so the
                     per-head S^T matmuls contract over 128 partitions at
                     full rate (the old tile_position row-packing ran the
                     two 64-row matmuls serially at ~2 cycles/row)
                 S^T = KTpad.T @ Q^T  [k, q] per (head, kc, qh)
                 P^T = exp(S^T)       bf16
                 O'^T = [V|1].T @ P^T [65, q]; row 64 = Z
                 1/Z via reciprocal_approx_fast on the Z row, broadcast
                     across partitions with a DRAM-bounce DMA
                 O^T = O'^T[0:64] * (1/Z)
                 y^T = pT.T @ O^T + b [C, N]
  host returns   y = yT.T per batch.

Engine split: PE matmuls; ACT exp; DVE reciprocal+normalize muls+QK casts;
Pool (nc.gpsimd) PSUM evacuation of O' and V, bias adds; DMA issue spread
over sync/scalar/vector/gpsimd queues so no single queue serializes.
"""

from collections import deque

import numpy as np

import concourse.bass as bass
import concourse.mybir as mybir
import concourse.tile as tile
from concourse import bacc
from concourse.bass_utils import run_bass_kernel_spmd

f32 = mybir.dt.float32
f32r = mybir.dt.float32r
bf16 = mybir.dt.bfloat16
EXP = mybir.ActivationFunctionType.Exp

B, N, C = 8, 1024, 1024
H, HD = 16, 64
SCALE = HD ** -0.5
NCORES = 8


def build():
    nc = bacc.Bacc("TRN2", target_bir_lowering=False, debug=False)
    xTb = nc.dram_tensor("xTb", [C, N], bf16, kind="ExternalInput")
    wqk = nc.dram_tensor("wqk", [16 * 128, 1024], bf16,
                         kind="ExternalInput")
    wvd = nc.dram_tensor("wvd", [2 * 128, 4096], bf16,
                         kind="ExternalInput")
    wpd = nc.dram_tensor("wpd", [8 * 128, 1024], bf16,
                         kind="ExternalInput")
    pb = nc.dram_tensor("pb", [C, 1], f32, kind="ExternalInput")
    yT = nc.dram_tensor("yT", [C, N], f32, kind="ExternalOutput")

    with tile.TileContext(nc) as tc:
        with (
            tc.tile_pool(name="const", bufs=1) as const,
            tc.tile_pool(name="xp", bufs=8) as xp,
            tc.tile_pool(name="vpp", bufs=8) as vpp,
            tc.tile_pool(name="obp", bufs=8) as obp,
            tc.tile_pool(name="qtp", bufs=2) as qtp,
            tc.tile_pool(name="wqkp", bufs=4) as wqkp,
            tc.tile_pool(name="wpp", bufs=8) as wpp,
            tc.tile_pool(name="ptsp", bufs=12) as ptsp,
            tc.tile_pool(name="ovsp", bufs=4) as ovsp,
            tc.tile_pool(name="rzp", bufs=2) as rzp,
            tc.tile_pool(name="otp", bufs=2) as otp,
            tc.tile_pool(name="ytp", bufs=2) as ytp,
            tc.tile_pool(name="psQ", bufs=1, space="PSUM") as psQ,
            tc.tile_pool(name="rdr", bufs=8, space="DRAM") as rdr,
        ):
            # ---- bias: one DMA into [128, 8]; column e = chunk e's bias
            pbt = const.tile([128, 8], f32)
            nc.sync.dma_start(
                out=pbt, in_=pb.rearrange("(e p) x -> p (e x)", p=128))

            # ---- x tiles: xT row chunks, f32r
            xts = [xp.tile([128, N], bf16, name=f"xt{i}", tag="xt")
                   for i in range(8)]
            for ci in range(8):
                eng = nc.sync if ci == 0 else nc.gpsimd
                eng.dma_start(
                    out=xts[ci],
                    in_=xTb[ci * 128:(ci + 1) * 128, :])

            # ---- V resident: [n-chunk][128, 16 heads, 64+1] bf16
            vp = [vpp.tile([128, H, HD + 1], bf16, name=f"vp{i}", tag="vp")
                  for i in range(8)]
            for n in range(8):
                nc.vector.memset(vp[n][:, :, HD:HD + 1], 1.0)

            # ---- O^T resident: tile hq = rows [hq*128,(hq+1)*128) of O^T
            ob = [obp.tile([128, N], bf16, name=f"ob{i}", tag="ob")
                  for i in range(8)]

            # ---- proj weights: tiles now, DMAs staggered one per pair start
            wpts = [wpp.tile([128, N], bf16, name=f"wpt{e}", tag="wpt")
                    for e in range(8)]

            def wpt_dma(e):
                nc.scalar.dma_start(
                    out=wpts[e], in_=wpd[e * 128:(e + 1) * 128, :])

            # ---- KTpad stationaries: two rotating sets, dead halves zeroed
            # once (never rewritten, so zeros persist across pair reuse)
            ktpA = [const.tile([128, N], f32r, name=f"ktpA{i}")
                    for i in range(2)]
            ktpB = [const.tile([128, N], f32r, name=f"ktpB{i}")
                    for i in range(2)]
            zc = const.tile([128, 512], f32, name="zc")
            nc.vector.memset(zc, 0.0)
            for i in range(2):
                for qh in range(2):
                    sl = slice(qh * 512, (qh + 1) * 512)
                    nc.vector.tensor_copy(ktpA[i][64:128, sl],
                                          zc[64:128, :])
                    nc.vector.tensor_copy(ktpB[i][0:64, sl], zc[0:64, :])

            # ---- qk projection chunk -> psQ accumulation (thunk-based)
            def qk_mm_thunks(d, dma_eng):
                """DMA the wT column chunk d now; return matmul thunks that
                accumulate [128, N] into a fresh psQ tile + the acc tile."""
                wt = wqkp.tile([128, N], bf16, name=f"wt{d}", tag="wt")
                dma_eng.dma_start(
                    out=wt, in_=wqk[d * 128:(d + 1) * 128, :])
                acc = psQ.tile([128, N], f32, name=f"qacc{d}", tag="qacc")
                thunks = []
                for ci in range(8):
                    for qh in range(2):
                        sl = slice(qh * 512, (qh + 1) * 512)

                        def t(ci=ci, sl=sl, acc=acc, wt=wt):
                            nc.tensor.matmul(
                                acc[:, sl], wt[:, ci * 128:(ci + 1) * 128],
                                xts[ci][:, sl],
                                start=(ci == 0), stop=(ci == 7))
                        thunks.append(t)
                return acc, thunks

            fill = deque()

            def pump(k):
                for _ in range(k):
                    if fill:
                        fill.popleft()()

            def flush():
                while fill:
                    fill.popleft()()

            # ---- startup: pair-0 Q and K chunks emitted directly
            accQ, tQ = qk_mm_thunks(0, nc.sync)
            for t in tQ:
                t()
            qt0 = qtp.tile([128, N], f32r, name="qt0", tag="qt")
            nc.vector.tensor_copy(qt0, accQ)
            accK, tK = qk_mm_thunks(8, nc.sync)
            for t in tK:
                t()
            nc.vector.tensor_copy(ktpA[0][0:64, :], accK[0:64, :])
            nc.vector.tensor_copy(ktpB[0][64:128, :], accK[64:128, :])

            # ---- phase V: value projection (scoped PSUM pool)
            with (
                tc.tile_pool(name="psV", bufs=3, space="PSUM") as psV,
                tc.tile_pool(name="wvp", bufs=2) as wvp,
            ):
                for dv in range(2):
                    wv = wvp.tile([128, 4096], bf16,
                                  name=f"wv{dv}", tag="wv")
                    nc.gpsimd.dma_start(
                        out=wv, in_=wvd[dv * 128:(dv + 1) * 128, :])
                    for n in range(8):
                        vacc = psV.tile([128, 512], f32, name="vacc",
                                        tag="vacc")
                        for ci in range(8):
                            nc.tensor.matmul(
                                vacc[:, :],
                                xts[ci][:, n * 128:(n + 1) * 128],
                                wv[:, ci * 512:(ci + 1) * 512],
                                start=(ci == 0), stop=(ci == 7))
                        nc.scalar.activation(
                            vp[n][:, dv * 8:(dv + 1) * 8, 0:HD],
                            vacc.rearrange("p (g e) -> p g e", e=HD),
                            mybir.ActivationFunctionType.Copy)

            # ---- attention pairs ------------------------------------
            with (
                tc.tile_pool(name="psS", bufs=2, space="PSUM") as psS,
                tc.tile_pool(name="psO", bufs=4, space="PSUM") as psO,
            ):
                def av_mm(ovt, h, kc, qh, pts):
                    sl = slice(qh * 512, (qh + 1) * 512)
                    nc.tensor.matmul(
                        ovt[:, :], vp[kc][:, h, :], pts[kc][:, sl],
                        start=(kc == 0), stop=(kc == 7))

                def norm(h, ov2, tail=False):
                    """O = O' * 1/Z for head h; writes ob in the right rows."""
                    hq, odd = h // 2, h % 2
                    rz = rzp.tile([64, N], f32, name=f"rz{h}", tag="rz")
                    ovss = []
                    for qh in range(2):
                        ovs = ovsp.tile([65, 512], f32, name=f"ovs{h}_{qh}",
                                        tag="ovs")
                        if tail:
                            nc.scalar.activation(
                                ovs, ov2[qh],
                                mybir.ActivationFunctionType.Copy)
                        else:
                            nc.vector.tensor_copy(ovs, ov2[qh])
                        zdr = rdr.tile([1, 512], f32, name="zd", tag="zd")
                        # write + broadcast-read share the sync queue: the
                        # DRAM bounce tile is not dep-tracked, FIFO order is
                        nc.sync.dma_start(out=zdr, in_=ovs[64:65, :])
                        bsrc = bass.AP(tensor=zdr.tensor, offset=zdr.offset,
                                       ap=[[0, 64], [1, 512]])
                        nc.sync.dma_start(
                            out=rz[:, qh * 512:(qh + 1) * 512], in_=bsrc)
                        # approx-fast reciprocal only works at
                        # partition base 0, so invert the broadcast copy
                        nc.vector.reciprocal_approx_fast(
                            rz[:, qh * 512:(qh + 1) * 512],
                            rz[:, qh * 512:(qh + 1) * 512])
                        ovss.append(ovs)
                    if not odd:
                        for qh in range(2):
                            sl = slice(qh * 512, (qh + 1) * 512)
                            nc.vector.tensor_mul(
                                ob[hq][0:64, sl], ovss[qh][0:64, :],
                                rz[:, sl])
                    else:
                        ot = otp.tile([64, N], bf16, name=f"ot{h}", tag="ot")
                        for qh in range(2):
                            sl = slice(qh * 512, (qh + 1) * 512)
                            nc.vector.tensor_mul(
                                ot[:, sl], ovss[qh][0:64, :], rz[:, sl])
                        nc.gpsimd.dma_start(out=ob[hq][64:128, :],
                                            in_=ot[:, :])

                QT = qt0
                ktA, ktB = ktpA[0], ktpB[0]

                # proj helpers (emitted at/after pair 7)
                def proj_mm(pjq, e, d, qh):
                    sl = slice(qh * 512, (qh + 1) * 512)
                    nc.tensor.matmul(
                        pjq[qh][:, :] if isinstance(pjq, list)
                        else pjq[:, sl],
                        wpts[e][:, d * 128:(d + 1) * 128], ob[d][:, sl],
                        start=(d == 0), stop=(d == 7))

                def proj_drain(pj, e):
                    yt = ytp.tile([128, N], f32, name=f"yt{e}", tag="yt")
                    for qh in range(2):
                        sl = slice(qh * 512, (qh + 1) * 512)
                        src_ = pj[qh][:, :] if isinstance(pj, list) \
                            else pj[:, sl]
                        nc.vector.tensor_scalar_add(
                            yt[:, sl], src_, pbt[:, e:e + 1])
                        nc.gpsimd.dma_start(
                            out=yT[e * 128:(e + 1) * 128, sl],
                            in_=yt[:, sl])

                pj0 = None
                for j in range(8):
                    hE, hO = 2 * j, 2 * j + 1
                    wpt_dma(j)
                    ptsE = [ptsp.tile([128, N], bf16, name=f"pe{j}_{k}",
                                      tag="pts") for k in range(8)]
                    ptsO = [ptsp.tile([128, N], bf16, name=f"po{j}_{k}",
                                      tag="pts") for k in range(8)]
                    # queue next pair's qk work as PE filler
                    if j < 7:
                        accQn, tQn = qk_mm_thunks(j + 1, nc.scalar)
                        qtn = qtp.tile([128, N], f32r, name=f"qt{j+1}",
                                       tag="qt")
                        fill.extend(tQn)
                        fill.append(lambda q=qtn, a=accQn:
                                    nc.vector.tensor_copy(q, a))
                        accKn, tKn = qk_mm_thunks(9 + j, nc.scalar)
                        fill.extend(tKn)
                        s = (j + 1) % 2

                        def kcast(a=accKn, s=s):
                            nc.vector.tensor_copy(ktpA[s][0:64, :],
                                                  a[0:64, :])
                            nc.vector.tensor_copy(ktpB[s][64:128, :],
                                                  a[64:128, :])
                        fill.append(kcast)
                        QTn = qtn
                    else:
                        # pair 7: chunk e=0 proj partials (d=0..6) as filler
                        pj0 = psQ.tile([128, N], f32, name="pj0", tag="qacc")
                        for d in range(7):
                            for qh in range(2):
                                fill.append(lambda d=d, qh=qh:
                                            proj_mm(pj0, 0, d, qh))

                    ovE = [psO.tile([65, 512], f32, name=f"ovE{j}_{q}",
                                    tag="ov") for q in range(2)]
                    ovO = [psO.tile([65, 512], f32, name=f"ovO{j}_{q}",
                                    tag="ov") for q in range(2)]

                    for kc in range(8):
                        ks = slice(kc * 128, (kc + 1) * 128)
                        for qh in range(2):
                            sl = slice(qh * 512, (qh + 1) * 512)
                            st = psS.tile([128, 512], f32, name="st",
                                          tag="st")
                            nc.tensor.matmul(st[:, :], ktA[:, ks],
                                             QT[:, sl], start=True,
                                             stop=True)
                            nc.scalar.activation(ptsE[kc][:, sl], st[:, :],
                                                 EXP)
                        pump(2)
                        if kc >= 1:
                            for qh in range(2):
                                av_mm(ovE[qh], hE, kc - 1, qh, ptsE)
                            pump(1)
                        for qh in range(2):
                            sl = slice(qh * 512, (qh + 1) * 512)
                            st = psS.tile([128, 512], f32, name="st",
                                          tag="st")
                            nc.tensor.matmul(st[:, :], ktB[:, ks],
                                             QT[:, sl], start=True,
                                             stop=True)
                            nc.scalar.activation(ptsO[kc][:, sl], st[:, :],
                                                 EXP)
                        pump(2)
                        if kc >= 1:
                            for qh in range(2):
                                av_mm(ovO[qh], hO, kc - 1, qh, ptsO)
                            pump(1)
                    # last AV chunk + normalization chains
                    for qh in range(2):
                        av_mm(ovE[qh], hE, 7, qh, ptsE)
                    if j == 7:
                        # tail: odd head first (its chain has the extra
                        # partition-shift DMA hop gating ob[7])
                        for qh in range(2):
                            av_mm(ovO[qh], hO, 7, qh, ptsO)
                        norm(hO, ovO, tail=True)
                        norm(hE, ovE, tail=True)
                    else:
                        norm(hE, ovE)
                        for qh in range(2):
                            av_mm(ovO[qh], hO, 7, qh, ptsO)
                        norm(hO, ovO)
                    if j < 7:
                        # leftover fillers include the next pair's KT cast;
                        # they must be emitted before pair j+1 reads ktA/ktB
                        flush()
                        QT, ktA, ktB = QTn, ktpA[(j + 1) % 2], \
                            ktpB[(j + 1) % 2]

                flush()
                # e1 partials cover the last norm-chain latency before the
                # ob[7]-gated d=7 taps
                pj1 = [psS.tile([128, 512], f32, name=f"pj1_{q}", tag="st")
                       for q in range(2)]
                for d in range(7):
                    for qh in range(2):
                        proj_mm(pj1, 1, d, qh)
                for qh in range(2):
                    proj_mm(pj0, 0, 7, qh)
                proj_drain(pj0, 0)
                for qh in range(2):
                    proj_mm(pj1, 1, 7, qh)
                proj_drain(pj1, 1)
                # chunks 2..7, two in flight (psQ tile / psS tile pair)
                for e in range(2, 8):
                    if e % 2 == 0:
                        pj = psQ.tile([128, N], f32, name=f"pj{e}",
                                      tag="qacc")
                    else:
                        pj = [psS.tile([128, 512], f32, name=f"pj{e}_{q}",
                                       tag="st") for q in range(2)]
                    for d in range(8):
                        for qh in range(2):
                            proj_mm(pj, e, d, qh)
                    proj_drain(pj, e)
    nc.compile()
    return nc


_CACHE = {}


def _get_nc():
    if "nc" not in _CACHE:
        _CACHE["nc"] = build()
    return _CACHE["nc"]


def _pack_chunks(w, ncol):
    """[C, k*ncol] -> [k*128, 8*ncol]: chunk d rows = SBUF layout
    [p, j*ncol + c] = w[j*128 + p, d*ncol + c]."""
    k = w.shape[1] // ncol
    out = np.empty((k * 128, 8 * ncol), dtype=w.dtype)
    for d in range(k):
        blk = w[:, d * ncol:(d + 1) * ncol].reshape(8, 128, ncol)
        out[d * 128:(d + 1) * 128] = (
            blk.transpose(1, 0, 2).reshape(128, 8 * ncol))
    return out


def _prep_in_maps(x, qkv_w, proj_w, proj_b):
    import ml_dtypes
    wT = np.ascontiguousarray(np.asarray(qkv_w, dtype=np.float32).T).copy()
    wT[:, 0:C] *= np.float32(SCALE)
    wTb = wT.astype(ml_dtypes.bfloat16)
    pTb = np.ascontiguousarray(np.asarray(proj_w, dtype=np.float32).T) \
        .astype(ml_dtypes.bfloat16)
    pbv = np.ascontiguousarray(np.asarray(proj_b, dtype=np.float32)
                               .reshape(C, 1))
    wqk = np.ascontiguousarray(_pack_chunks(wTb[:, 0:2048], 128))
    wvd = np.ascontiguousarray(_pack_chunks(wTb[:, 2048:3072], 512))
    wpd = np.ascontiguousarray(_pack_chunks(pTb, 128))
    return [
        {
            "xTb": np.ascontiguousarray(np.asarray(x[b], dtype=np.float32).T)
            .astype(ml_dtypes.bfloat16),
            "wqk": wqk,
            "wvd": wvd,
            "wpd": wpd,
            "pb": pbv,
        }
        for b in range(B)
    ]


def kernel(x, qkv_w, proj_w, proj_b):
    x = np.asarray(x)
    assert x.shape == (B, N, C), x.shape
    nc = _get_nc()
    in_maps = _prep_in_maps(x, qkv_w, proj_w, proj_b)
    res = run_bass_kernel_spmd(nc, in_maps, core_ids=list(range(NCORES)))
    out = np.stack([res.results[b]["yT"].T for b in range(B)], axis=0)
    return np.ascontiguousarray(out.astype(np.float32))
